# revision 5
# baseline (speedup 1.0000x reference)
"""Causal multi-head attention on 8 TRN2 NeuronCores.

Problem: B=4, T=2048, C=1024, H=16 heads, D=64. f32 in/out.

Sharding (tensor parallel over heads x batch): core i = (b = i//2, g = i%2)
handles batch b and head-group g (8 heads = 512 channels).  Each core gets
  xt  = x[b].T                      [C, T]   (pre-transposed on host)
  wq/wk/wv = w_qkv column slices    [C, 512]
  wp  = w_proj row slice            [512, C]
and produces a PARTIAL projection output out^T [C, T]; the host sums the two
group partials per batch and transposes back.  No on-device collectives.

Per-core pipeline (all matmuls bf16, f32 accumulation):
  A) Q^T,K^T = (w_q|w_k)^T-stationary matmuls vs x^T;  V in natural [T,D]
     layout with a ones-column appended (V_aug) per head.
  B) S^T[k,q] = K^T-stationary matmul (two heads row-packed on the 128-row
     PE array), exp via ScalarE with fused 1/sqrt(D) scale, causal mask via
     gpsimd affine_select, AV via V_aug-stationary matmul -> Y^T with the
     softmax denominator Z landing in row 64 for free.  Normalize with DVE
     reciprocal + DMA partition-broadcast + DVE multiply.
  C) out^T[c,q] = w_proj-stationary matmul vs Y^T, DMA out.
"""

import numpy as np

B, T, C, H, D = 4, 2048, 1024, 16, 64
G = 2          # head groups (cores per batch)
GC = 512       # channels per group (8 heads * 64)
NCORES = 8
CT = C // 128   # 8 c-tiles
NT = T // 128   # 16 t-tiles of 128
TB = T // 512   # 4 t-blocks of 512
HP = 4          # head-pairs per group

_CACHE = {}


def _build():
    import concourse.bass as bass
    import concourse.tile as tile
    from concourse import bacc, mybir

    f32 = mybir.dt.float32
    bf16 = mybir.dt.bfloat16
    Alu = mybir.AluOpType
    Act = mybir.ActivationFunctionType

    nc = bacc.Bacc("TRN2", target_bir_lowering=False, debug=False,
                   num_devices=NCORES)

    xt = nc.dram_tensor("xt", [C, T], f32, kind="ExternalInput").ap()
    wq = nc.dram_tensor("wq", [C, GC], f32, kind="ExternalInput").ap()
    wk = nc.dram_tensor("wk", [C, GC], f32, kind="ExternalInput").ap()
    wv = nc.dram_tensor("wv", [C, GC], f32, kind="ExternalInput").ap()
    wp = nc.dram_tensor("wp", [GC, C], f32, kind="ExternalInput").ap()
    out = nc.dram_tensor("out", [C, T], f32, kind="ExternalOutput").ap()

    xt3 = xt.rearrange("(co p) t -> p co t", p=128)     # [128, 8, T]
    wq3 = wq.rearrange("(co p) n -> p co n", p=128)     # [128, 8, 512]
    wk3 = wk.rearrange("(co p) n -> p co n", p=128)
    wv3 = wv.rearrange("(co p) n -> p co n", p=128)
    wp3 = wp.rearrange("(yo p) n -> p yo n", p=128)     # [128, 4, 1024]
    out3 = out.rearrange("(co p) t -> p co t", p=128)   # [128, 8, T]

    with tile.TileContext(nc) as tc:
        with tc.tile_pool(name="persist", bufs=1) as persist:
            # persistent SBUF tensors (per-partition KB in comments)
            xbf = persist.tile([128, CT, T], bf16)        # 32K
            wqb = persist.tile([128, CT, GC], bf16)       # 8K
            wkb = persist.tile([128, CT, GC], bf16)       # 8K
            wvb = persist.tile([128, CT, GC], bf16)       # 8K
            qt = persist.tile([128, HP, T], bf16)         # 16K
            kt = persist.tile([128, HP, T], bf16)         # 16K
            vsb = persist.tile([128, 8, NT, 65], bf16)    # 16.3K
            yt = persist.tile([128, 4, T], bf16)          # 16K
            wpb = persist.tile([128, 4, C], bf16)         # 8K

            # ---------------- Phase A: load, cast, QKV projections ------
            with tc.tile_pool(name="stg", bufs=2) as stg, \
                 tc.tile_pool(name="psA", bufs=4, space="PSUM") as psA:
                # weights: load f32, cast to bf16
                for wsrc, wdst in ((wq3, wqb), (wk3, wkb), (wv3, wvb)):
                    s = stg.tile([128, CT, GC], f32, tag="stg")
                    nc.sync.dma_start(out=s, in_=wsrc)
                    nc.vector.tensor_copy(out=wdst, in_=s)
                sp = stg.tile([128, 4, C], f32, tag="stg")
                nc.sync.dma_start(out=sp, in_=wp3)
                nc.vector.tensor_copy(out=wpb, in_=sp)

                # ones column of V_aug
                nc.vector.memset(vsb[:, :, :, 64:65], 1.0)

                for tb in range(TB):
                    tsl = slice(tb * 512, tb * 512 + 512)
                    xs = stg.tile([128, CT, 512], f32, tag="stg")
                    nc.sync.dma_start(out=xs, in_=xt3[:, :, tsl])
                    nc.vector.tensor_copy(out=xbf[:, :, tsl], in_=xs)

                    # Q^T and K^T: [ch, t] = w-stationary vs x^T
                    for wsb, dst in ((wqb, qt), (wkb, kt)):
                        for hp in range(HP):
                            ps = psA.tile([128, 512], f32, tag="psA")
                            for c in range(CT):
                                nc.tensor.matmul(
                                    out=ps,
                                    lhsT=wsb[:, c, hp * 128:hp * 128 + 128],
                                    rhs=xbf[:, c, tsl],
                                    start=(c == 0), stop=(c == CT - 1))
                            nc.scalar.copy(out=dst[:, hp, tsl], in_=ps)

                    # V natural layout: [t, ch] = x^T-stationary vs w_v
                    for tj in range(tb * 4, tb * 4 + 4):
                        ps = psA.tile([128, 512], f32, tag="psA")
                        for c in range(CT):
                            nc.tensor.matmul(
                                out=ps,
                                lhsT=xbf[:, c, tj * 128:tj * 128 + 128],
                                rhs=wvb[:, c, :],
                                start=(c == 0), stop=(c == CT - 1))
                        nc.scalar.copy(
                            out=vsb[:, :, tj, 0:64],
                            in_=ps.rearrange("p (h d) -> p h d", h=8))

            # ---------------- Phase B: attention ------------------------
            with tc.tile_pool(name="ptp", bufs=6) as ptp, \
                 tc.tile_pool(name="smal", bufs=8) as smal, \
                 tc.tile_pool(name="dramp", bufs=4, space="DRAM") as dramp, \
                 tc.tile_pool(name="yap", bufs=4, space="PSUM") as yap, \
                 tc.tile_pool(name="stp", bufs=4, space="PSUM") as stp:
                for hp in range(HP):
                    for qb in range(TB):
                        qsl = slice(qb * 512, qb * 512 + 512)
                        nk = 4 * qb + 4
                        ya = [yap.tile([65, 512], f32, tag="yap", name=f"ya{_h}")
                              for _h in range(2)]
                        for j in range(nk):
                            off = j - 4 * qb  # >=0 on diagonal k-tiles
                            for h2 in range(2):
                                p0 = 64 * h2
                                st = stp.tile([128, 512], f32, tag="stp")
                                nc.tensor.matmul(
                                    out=st,
                                    lhsT=kt[p0:p0 + 64, hp,
                                            j * 128:j * 128 + 128],
                                    rhs=qt[p0:p0 + 64, hp, qsl],
                                    start=True, stop=True,
                                    tile_position=(p0, 0))
                                pt = ptp.tile([128, 512], bf16, tag="ptp")
                                if off <= 0:
                                    nc.scalar.activation(
                                        out=pt, in_=st, func=Act.Exp,
                                        scale=0.125)
                                else:
                                    # leading 128*off cols are fully masked
                                    nc.gpsimd.memset(pt[:, 0:128 * off], 0.0)
                                    nc.scalar.activation(
                                        out=pt[:, 128 * off:],
                                        in_=st[:, 128 * off:],
                                        func=Act.Exp, scale=0.125)
                                if off >= 0:
                                    # triangular mask on the diagonal block:
                                    # keep where q_local >= k_local
                                    d0 = 128 * off
                                    nc.gpsimd.affine_select(
                                        out=pt[:, d0:d0 + 128],
                                        in_=pt[:, d0:d0 + 128],
                                        pattern=[[1, 128]],
                                        compare_op=Alu.is_ge,
                                        fill=0.0,
                                        base=0,
                                        channel_multiplier=-1)
                                nc.tensor.matmul(
                                    out=ya[h2],
                                    lhsT=vsb[:, 2 * hp + h2, j, :],
                                    rhs=pt,
                                    start=(j == 0), stop=(j == nk - 1))
                        for h2 in range(2):
                            r = smal.tile([1, 512], f32, tag="r")
                            nc.vector.reciprocal(r, ya[h2][64:65, :])
                            rd = dramp.tile([1, 512], f32, tag="rd")
                            nc.gpsimd.dma_start(out=rd, in_=r)
                            rb = smal.tile([64, 512], f32, tag="rb")
                            nc.gpsimd.dma_start(
                                out=rb, in_=rd.to_broadcast([64, 512]))
                            nc.vector.tensor_mul(
                                out=yt[64 * h2:64 * h2 + 64, hp, qsl],
                                in0=ya[h2][0:64, :],
                                in1=rb)

            # ---------------- Phase C: output projection ----------------
            with tc.tile_pool(name="psC", bufs=4, space="PSUM") as psC, \
                 tc.tile_pool(name="ostg", bufs=4) as ostg:
                for co in range(CT):
                    pso = [psC.tile([128, 512], f32, tag="psC", name=f"pso{_q}")
                           for _q in range(TB)]
                    for yti in range(4):
                        for qb in range(TB):
                            nc.tensor.matmul(
                                out=pso[qb],
                                lhsT=wpb[:, yti, co * 128:co * 128 + 128],
                                rhs=yt[:, yti, qb * 512:qb * 512 + 512],
                                start=(yti == 0), stop=(yti == 3))
                    for qb in range(TB):
                        ob = ostg.tile([128, 512], f32, tag="ostg")
                        nc.scalar.copy(out=ob, in_=pso[qb])
                        nc.sync.dma_start(
                            out=out3[:, co, qb * 512:qb * 512 + 512],
                            in_=ob)

    nc.compile()
    return nc


def _get_nc():
    if "nc" not in _CACHE:
        _CACHE["nc"] = _build()
    return _CACHE["nc"]


def _make_in_maps(x, w_qkv, w_proj):
    x = np.asarray(x, dtype=np.float32)
    w_qkv = np.asarray(w_qkv, dtype=np.float32)
    w_proj = np.asarray(w_proj, dtype=np.float32)
    in_maps = []
    for i in range(NCORES):
        b, g = divmod(i, G)
        cs = slice(g * GC, (g + 1) * GC)
        in_maps.append({
            "xt": np.ascontiguousarray(x[b].T),
            "wq": np.ascontiguousarray(w_qkv[:, cs]),
            "wk": np.ascontiguousarray(w_qkv[:, C + g * GC:C + (g + 1) * GC]),
            "wv": np.ascontiguousarray(
                w_qkv[:, 2 * C + g * GC:2 * C + (g + 1) * GC]),
            "wp": np.ascontiguousarray(w_proj[cs, :]),
        })
    return in_maps


def _run(x, w_qkv, w_proj, trace=False):
    from concourse.bass_utils import run_bass_kernel_spmd
    nc = _get_nc()
    res = run_bass_kernel_spmd(nc, _make_in_maps(x, w_qkv, w_proj),
                               core_ids=list(range(NCORES)), trace=trace)
    outs = [np.asarray(r["out"], dtype=np.float32) for r in res.results]
    full = np.empty((B, T, C), dtype=np.float32)
    for b in range(B):
        full[b] = (outs[2 * b] + outs[2 * b + 1]).T
    return full, res


def kernel(x, w_qkv, w_proj):
    full, _ = _run(x, w_qkv, w_proj, trace=False)
    return full


def _install_trace_shims():
    """The agent image lacks antenv.axon_hooks; recreate the NTFF hook the
    axon boot would have registered, and skip the artifact upload (no
    network egress here)."""
    import sys
    import types

    import antenv
    from concourse import bass_utils

    bass_utils.upload_artifacts = lambda tmpdir: tmpdir
    if "antenv.axon_hooks" not in sys.modules:
        import os as _os

        from trn_agent_boot import trn_boot
        hook = trn_boot._ntff_profile_via_ctypes(
            _os.environ.get("PJRT_LIBRARY_PATH", "/opt/axon/libaxon_pjrt.so"))
        mod = types.ModuleType("antenv.axon_hooks")
        mod.get_axon_ntff_profile_hook = lambda: hook
        mod.set_axon_ntff_profile_hook = lambda h: None
        sys.modules["antenv.axon_hooks"] = mod
        antenv.axon_hooks = mod


def bench(x, w_qkv, w_proj):
    """Returns (output, exec_time_ns)."""
    _install_trace_shims()
    full, res = _run(x, w_qkv, w_proj, trace=True)
    return full, res.exec_time_ns


# revision 12
# speedup vs baseline: 1.1836x; 1.1836x over previous
"""Causal multi-head attention on 8 TRN2 NeuronCores.

Problem: B=4, T=2048, C=1024, H=16 heads, D=64. f32 in/out.

Sharding (tensor parallel over heads x batch): core i = (b = i//2, g = i%2)
handles batch b and head-group g (8 heads = 512 channels).  Each core gets
  xt  = x[b].T                      [C, T]   (pre-transposed on host)
  wq/wk/wv = w_qkv column slices    [C, 512]
  wp  = w_proj row slice            [512, C]
and produces a PARTIAL projection output out^T [C, T]; the host sums the two
group partials per batch and transposes back.  No on-device collectives.

Per-core macro-pipeline over t-blocks of 512 (causality makes attention for
query block qb depend only on K/V t-blocks <= qb):
  A(tb): load x^T chunk, cast to bf16, project Q^T,K^T (w-stationary) and
         V (x^T-stationary, natural layout, ones-column appended per head).
  B(qb=tb): per head-pair: S^T[k,q] matmuls (2 heads row-packed via
         tile_position), exp on ScalarE with fused 1/8 scale (valid columns
         only), causal triangle mask via gpsimd affine_select, AV matmuls
         against V_aug -> Y^T with softmax denominator Z in row 64 for free.
         Z rows collect into one [8,512] tile; one exact DVE reciprocal per
         q-block; 1/Z broadcast across partitions via a DRAM round-trip DMA;
         final normalize multiplies write bf16 Y^T.
  C(qb=tb): out^T tile = w_proj-stationary matmul vs Y^T, DVE copy, DMA out.
"""

import numpy as np

B, T, C, H, D = 4, 2048, 1024, 16, 64
G = 2          # head groups (cores per batch)
GC = 512       # channels per group (8 heads * 64)
NCORES = 8
CT = C // 128   # 8 c-tiles
NT = T // 128   # 16 t-tiles of 128
TB = T // 512   # 4 t-blocks of 512
HP = 4          # head-pairs per group

_CACHE = {}


def _build():
    import concourse.bass as bass
    import concourse.tile as tile
    from concourse import bacc, mybir

    f32 = mybir.dt.float32
    bf16 = mybir.dt.bfloat16
    Alu = mybir.AluOpType
    Act = mybir.ActivationFunctionType

    nc = bacc.Bacc("TRN2", target_bir_lowering=False, debug=False,
                   num_devices=NCORES)

    xt = nc.dram_tensor("xt", [C, T], f32, kind="ExternalInput").ap()
    wq = nc.dram_tensor("wq", [C, GC], f32, kind="ExternalInput").ap()
    wk = nc.dram_tensor("wk", [C, GC], f32, kind="ExternalInput").ap()
    wv = nc.dram_tensor("wv", [C, GC], f32, kind="ExternalInput").ap()
    wp = nc.dram_tensor("wp", [GC, C], f32, kind="ExternalInput").ap()
    out = nc.dram_tensor("out", [C, T], f32, kind="ExternalOutput").ap()

    xt3 = xt.rearrange("(co p) t -> p co t", p=128)     # [128, 8, T]
    wq3 = wq.rearrange("(co p) n -> p co n", p=128)     # [128, 8, 512]
    wk3 = wk.rearrange("(co p) n -> p co n", p=128)
    wv3 = wv.rearrange("(co p) n -> p co n", p=128)
    wp3 = wp.rearrange("(yo p) n -> p yo n", p=128)     # [128, 4, 1024]
    out3 = out.rearrange("(co p) t -> p co t", p=128)   # [128, 8, T]

    with tile.TileContext(nc) as tc:
        with tc.tile_pool(name="persist", bufs=1) as persist, \
             tc.tile_pool(name="stg", bufs=2) as stg, \
             tc.tile_pool(name="ptp", bufs=6) as ptp, \
             tc.tile_pool(name="smal", bufs=4) as smal, \
             tc.tile_pool(name="yub", bufs=2) as yubp, \
             tc.tile_pool(name="ostg", bufs=3) as ostg, \
             tc.tile_pool(name="dramp", bufs=2, space="DRAM") as dramp, \
             tc.tile_pool(name="psA", bufs=2, space="PSUM") as psA, \
             tc.tile_pool(name="stc", bufs=3, space="PSUM") as stc, \
             tc.tile_pool(name="yap", bufs=3, space="PSUM") as yap:
            # persistent SBUF tensors (per-partition KB in comments)
            xbf = persist.tile([128, CT, T], bf16)        # 32K
            wqb = persist.tile([128, CT, GC], bf16)       # 8K
            wkb = persist.tile([128, CT, GC], bf16)       # 8K
            wvb = persist.tile([128, CT, GC], bf16)       # 8K
            qt = persist.tile([128, HP, T], bf16)         # 16K
            kt = persist.tile([128, HP, T], bf16)         # 16K
            vsb = persist.tile([128, 8, NT, 65], bf16)    # 16.3K
            yt = persist.tile([128, 4, T], bf16)          # 16K
            wpb = persist.tile([128, 4, C], bf16)         # 8K

            def load_cast(dst, src, ncols, nchunk):
                # stream src -> f32 staging chunks -> bf16 dst
                step = ncols // nchunk
                for i in range(nchunk):
                    csl = slice(i * step, (i + 1) * step)
                    s = stg.tile([128, src.shape[1], step], f32, tag="stg",
                                 name="s")
                    nc.sync.dma_start(out=s, in_=src[:, :, csl])
                    nc.vector.tensor_copy(out=dst[:, :, csl], in_=s)

            # wq then first x chunk first so matmuls start ASAP
            load_cast(wqb, wq3, GC, 2)
            load_cast(xbf[:, :, 0:512], xt3[:, :, 0:512], 512, 2)
            load_cast(wkb, wk3, GC, 2)
            load_cast(wvb, wv3, GC, 2)
            # ones column of V_aug
            nc.vector.memset(vsb[:, :, :, 64:65], 1.0)

            wp_loaded = [False]

            def phase_a(tb):
                tsl = slice(tb * 512, tb * 512 + 512)
                # Q^T and K^T: [ch, t] = w-stationary vs x^T
                for wsb, dst in ((wqb, qt), (wkb, kt)):
                    for hp in range(HP):
                        ps = psA.tile([128, 512], f32, tag="psA", name="psA")
                        for c in range(CT):
                            nc.tensor.matmul(
                                out=ps,
                                lhsT=wsb[:, c, hp * 128:hp * 128 + 128],
                                rhs=xbf[:, c, tsl],
                                start=(c == 0), stop=(c == CT - 1))
                        nc.vector.tensor_copy(out=dst[:, hp, tsl], in_=ps)
                # V natural layout: [t, ch] = x^T-stationary vs w_v
                for tj in range(tb * 4, tb * 4 + 4):
                    ps = psA.tile([128, 512], f32, tag="psA", name="psV")
                    for c in range(CT):
                        nc.tensor.matmul(
                            out=ps,
                            lhsT=xbf[:, c, tj * 128:tj * 128 + 128],
                            rhs=wvb[:, c, :],
                            start=(c == 0), stop=(c == CT - 1))
                    nc.vector.tensor_copy(
                        out=vsb[:, :, tj, 0:64],
                        in_=ps.rearrange("p (h d) -> p h d", h=8))

            def phase_b(qb):
                qsl = slice(qb * 512, qb * 512 + 512)
                nk = 4 * qb + 4
                # Z rows for groups g=2*hp+h2 collect at partition 32*(g%4)
                # of zz[g//4]; one reciprocal covers 4 groups.
                zz = [smal.tile([128, 512], f32, tag="zz", name=f"zz{_i}")
                      for _i in range(2)]
                for _z in zz:
                    nc.gpsimd.memset(_z, 1.0)
                rrs = [smal.tile([128, 512], f32, tag="zz", name=f"rr{_i}")
                       for _i in range(2)]
                rds = [dramp.tile([4, 512], f32, tag="rd", name=f"rd{_i}")
                       for _i in range(2)]
                yub = yubp.tile([128, HP, 512], f32, tag="yub", name="yub")
                for hp in range(HP):
                    ya = [yap.tile([65, 512], f32, tag="yap", name=f"ya{_h}")
                          for _h in range(2)]
                    for j in range(nk):
                        off = j - 4 * qb  # >=0 on diagonal k-tiles
                        v0 = max(0, 128 * off)  # first causally-valid column
                        for h2 in range(2):
                            p0 = 64 * h2
                            st = stc.tile([128, 512], f32, tag="stc",
                                          name="st")
                            nc.tensor.matmul(
                                out=st[:, v0:],
                                lhsT=kt[p0:p0 + 64, hp,
                                        j * 128:j * 128 + 128],
                                rhs=qt[p0:p0 + 64, hp,
                                       qb * 512 + v0:qb * 512 + 512],
                                start=True, stop=True,
                                tile_position=(p0, 0))
                            pt = ptp.tile([128, 512], bf16, tag="ptp",
                                          name="pt")
                            nc.scalar.activation(
                                out=pt[:, v0:], in_=st[:, v0:],
                                func=Act.Exp, scale=0.125)
                            if off >= 0:
                                # triangular mask on the diagonal 128-block:
                                # keep where q_local >= k_local
                                nc.gpsimd.affine_select(
                                    out=pt[:, v0:v0 + 128],
                                    in_=pt[:, v0:v0 + 128],
                                    pattern=[[1, 128]],
                                    compare_op=Alu.is_ge,
                                    fill=0.0,
                                    base=0,
                                    channel_multiplier=-1)
                            nc.tensor.matmul(
                                out=ya[h2][:, v0:],
                                lhsT=vsb[:, 2 * hp + h2, j, :],
                                rhs=pt[:, v0:],
                                start=(j == 0), stop=(j == nk - 1),
                                skip_group_check=True)
                    for h2 in range(2):
                        g = 2 * hp + h2
                        row = 32 * (g % 4)
                        nc.vector.tensor_copy(
                            out=zz[g // 4][row:row + 1, :],
                            in_=ya[h2][64:65, :])
                        nc.vector.tensor_copy(
                            out=yub[64 * h2:64 * h2 + 64, hp, :],
                            in_=ya[h2][0:64, :])
                    if hp % 2 == 1:
                        # groups (2hp-2 .. 2hp+1) complete -> reciprocal +
                        # DRAM round-trip for the partition broadcast
                        i = hp // 2
                        nc.vector.reciprocal(rrs[i], zz[i])
                        nc.sync.dma_start(
                            out=rds[i],
                            in_=rrs[i].rearrange("(a b) n -> a b n",
                                                 b=32)[:, 0, :])
                for hp in range(HP):
                    i, g0, g1 = hp // 2, 2 * hp, 2 * hp + 1
                    rb = smal.tile([128, 512], f32, tag="rb", name="rb")
                    nc.sync.dma_start(
                        out=rb[0:64],
                        in_=rds[i][g0 % 4:g0 % 4 + 1].to_broadcast([64, 512]))
                    nc.sync.dma_start(
                        out=rb[64:128],
                        in_=rds[i][g1 % 4:g1 % 4 + 1].to_broadcast([64, 512]))
                    nc.vector.tensor_mul(
                        out=yt[:, hp, qsl],
                        in0=yub[:, hp, :],
                        in1=rb)

            def phase_c(qb):
                qsl = slice(qb * 512, qb * 512 + 512)
                for co in range(CT):
                    ps = stc.tile([128, 512], f32, tag="stc", name="psC")
                    for yti in range(4):
                        nc.tensor.matmul(
                            out=ps,
                            lhsT=wpb[:, yti, co * 128:co * 128 + 128],
                            rhs=yt[:, yti, qsl],
                            start=(yti == 0), stop=(yti == 3))
                    ob = ostg.tile([128, 512], f32, tag="ostg", name="ob")
                    nc.vector.tensor_copy(out=ob, in_=ps)
                    nc.sync.dma_start(out=out3[:, co, qsl], in_=ob)

            for tb in range(TB):
                if tb > 0:
                    load_cast(xbf[:, :, tb * 512:tb * 512 + 512],
                              xt3[:, :, tb * 512:tb * 512 + 512], 512, 2)
                phase_a(tb)
                phase_b(tb)
                if not wp_loaded[0]:
                    load_cast(wpb, wp3, C, 2)
                    wp_loaded[0] = True
                phase_c(tb)

    nc.compile()
    return nc


def _get_nc():
    if "nc" not in _CACHE:
        _CACHE["nc"] = _build()
    return _CACHE["nc"]


def _make_in_maps(x, w_qkv, w_proj):
    x = np.asarray(x, dtype=np.float32)
    w_qkv = np.asarray(w_qkv, dtype=np.float32)
    w_proj = np.asarray(w_proj, dtype=np.float32)
    in_maps = []
    for i in range(NCORES):
        b, g = divmod(i, G)
        cs = slice(g * GC, (g + 1) * GC)
        in_maps.append({
            "xt": np.ascontiguousarray(x[b].T),
            "wq": np.ascontiguousarray(w_qkv[:, cs]),
            "wk": np.ascontiguousarray(w_qkv[:, C + g * GC:C + (g + 1) * GC]),
            "wv": np.ascontiguousarray(
                w_qkv[:, 2 * C + g * GC:2 * C + (g + 1) * GC]),
            "wp": np.ascontiguousarray(w_proj[cs, :]),
        })
    return in_maps


def _run(x, w_qkv, w_proj, trace=False):
    from concourse.bass_utils import run_bass_kernel_spmd
    nc = _get_nc()
    res = run_bass_kernel_spmd(nc, _make_in_maps(x, w_qkv, w_proj),
                               core_ids=list(range(NCORES)), trace=trace)
    outs = [np.asarray(r["out"], dtype=np.float32) for r in res.results]
    full = np.empty((B, T, C), dtype=np.float32)
    for b in range(B):
        full[b] = (outs[2 * b] + outs[2 * b + 1]).T
    return full, res


def kernel(x, w_qkv, w_proj):
    full, _ = _run(x, w_qkv, w_proj, trace=False)
    return full


def _install_trace_shims():
    """The agent image lacks antenv.axon_hooks; recreate the NTFF hook the
    axon boot would have registered, and skip the artifact upload (no
    network egress here)."""
    import sys
    import types

    import antenv
    from concourse import bass_utils

    bass_utils.upload_artifacts = lambda tmpdir: tmpdir
    if "antenv.axon_hooks" not in sys.modules:
        import os as _os

        from trn_agent_boot import trn_boot
        hook = trn_boot._ntff_profile_via_ctypes(
            _os.environ.get("PJRT_LIBRARY_PATH", "/opt/axon/libaxon_pjrt.so"))
        mod = types.ModuleType("antenv.axon_hooks")
        mod.get_axon_ntff_profile_hook = lambda: hook
        mod.set_axon_ntff_profile_hook = lambda h: None
        sys.modules["antenv.axon_hooks"] = mod
        antenv.axon_hooks = mod


def bench(x, w_qkv, w_proj):
    """Returns (output, exec_time_ns)."""
    _install_trace_shims()
    full, res = _run(x, w_qkv, w_proj, trace=True)
    return full, res.exec_time_ns
